# revision 15
# baseline (speedup 1.0000x reference)
"""KGCompletionGNN Trainium2 kernel (8 NeuronCores, SPMD edge-sharding).

v2: bf16 matmuls (4x PE), batched wide DMAs (amortize SWDGE/HWDGE fixed
costs), message scatter into a block-padded pair-interleaved slot buffer
(full-rate contiguous phase-B reads, no phase-B gathers), pair-interleaved
bf16 aggregate with chunked ReduceScatter, wide vector/scalar ops.

Strategy (per sharding hint): shard edges across 8 cores, replicate H.
Per layer:
  Phase A (dbl-groups of 8 edge-chunks): one batched indirect-DMA gather for
    H[head]/H[tail], wide PE transposes, bf16 edge-update matmuls + LayerNorm
    -> E_new; message matmuls -> indirect-scatter bf16 messages into a
    host-zeroed, destination-sorted slot buffer (layout pairs two 128-slot
    windows per 512B DRAM row so phase B reads at full DMA rate).
  Phase B: contiguous reads of slot window-pairs, wide onehot build
    (one is_equal per 16 windows), onehot matmuls accumulate per-node-block
    PSUM quads -> pair-interleaved partial agg [n_pad, d] bf16.
  Chunked ReduceScatter -> per-core node shard; mean + leaky-relu + residual
  + LayerNorm -> H shard; AllGather bf16 H between layers.
Host does index preprocessing only (sharding, sorting, slot schedule).
"""

import sys

sys.path.insert(0, "/opt/trn_rl_repo")

import numpy as np
import ml_dtypes

BF16 = ml_dtypes.bfloat16
P = 128
G2 = 8          # chunks per dbl-group (phase A)
LRELU_SLOPE = 0.01
LN_EPS = 1e-5


# ---------------------------------------------------------------- host prep
def _prep_host(H, E, ht, params, ncores):
    n, d = H.shape
    m = E.shape[0]
    assert d == P
    m_loc = m // ncores
    a2c = -(-m_loc // (P * G2)) * G2            # chunks, multiple of G2
    m_pad2 = a2c * P
    n_pad = -(-n // (ncores * 2 * P)) * (ncores * 2 * P)
    nblocks = n_pad // P
    npairs = nblocks // 2
    L = params["W_eu"].shape[0]

    # ---- flags (biases / ln affine)
    flags = dict(
        beu=bool(np.any(params["b_eu"])), bf=bool(np.any(params["b_fwd"])),
        bb=bool(np.any(params["b_back"])),
        ge=bool(np.any(params["ln_e_g"] != 1)), be=bool(np.any(params["ln_e_b"])),
        gh=bool(np.any(params["ln_h_g"] != 1)), bh=bool(np.any(params["ln_h_b"])),
    )

    # ---- global slot schedule: combined fwd+back messages, dst-sorted,
    # per-block window counts maxed over cores (SPMD-uniform trip counts).
    heads = [ht[c * m_loc:(c + 1) * m_loc, 0].astype(np.int64) for c in range(ncores)]
    tails = [ht[c * m_loc:(c + 1) * m_loc, 1].astype(np.int64) for c in range(ncores)]
    counts = np.zeros((ncores, nblocks), np.int64)
    for c in range(ncores):
        dst = np.concatenate([tails[c], heads[c]])
        counts[c] = np.bincount(dst >> 7, minlength=nblocks)[:nblocks]
    k_b = np.maximum(1, -(-counts.max(axis=0) // P))
    T = int(k_b.sum())
    if T % 16:
        k_b[-1] += 16 - T % 16
        T += 16 - T % 16
    W0 = np.zeros(nblocks + 1, np.int64)
    W0[1:] = np.cumsum(k_b)
    wblk = np.repeat(np.arange(nblocks), k_b)     # block of each window
    R_slots = T * P
    n_trash = 2 * (m_pad2 - m_loc)
    R = R_slots + n_trash

    # slot s -> element-row r (pair-interleaved window layout)
    def r_of_slot(s):
        w = s >> 7
        return 2 * ((w >> 1) * P + (s & 127)) + (w & 1)

    # ---- RS chunking / node ownership
    CH = []
    base = 0
    nch = 8
    csz = max(ncores, (npairs // nch) // ncores * ncores)
    if npairs > nch * ncores:
        sizes = [csz] * (nch - 1) + [npairs - (nch - 1) * csz]
    else:
        sizes = [npairs]
    for ln_ in sizes:
        CH.append((base, ln_))
        base += ln_
    own_pairs = [[] for _ in range(ncores)]
    for (p0, ln_) in CH:
        per = ln_ // ncores
        for c in range(ncores):
            own_pairs[c].extend(range(p0 + c * per, p0 + (c + 1) * per))
    opc = len(own_pairs[0])                        # owned pairs per core
    shard_n = opc * 2 * P
    invperm = np.zeros(n_pad, np.int64)
    row = 0
    for c in range(ncores):
        for gp in own_pairs[c]:
            invperm[2 * gp * P:(2 * gp + 2) * P] = np.arange(row, row + 2 * P)
            row += 2 * P

    # ---- global count normalization
    cnt = (np.bincount(ht[:, 1], minlength=n_pad)
           + np.bincount(ht[:, 0], minlength=n_pad)).astype(np.float32)
    inv_cnt = (1.0 / np.maximum(cnt, 1.0)).astype(np.float32)

    H_pad = np.zeros((n_pad, d), np.float32)
    H_pad[:n] = H

    meta = dict(
        n=n, d=d, m=m, m_loc=m_loc, a2c=a2c, m_pad2=m_pad2, n_pad=n_pad,
        nblocks=nblocks, npairs=npairs, L=L, T=T, R=R, ncores=ncores,
        k_b=k_b, W0=W0, wblk=wblk, CH=CH, opc=opc, shard_n=shard_n,
        flags=flags, own_pairs=own_pairs,
    )

    def t128(ix, fill=0):
        out = np.full(m_pad2, fill, np.int64)
        out[:len(ix)] = ix
        return np.ascontiguousarray(out.reshape(a2c, P).T).astype(np.int32)

    iota = np.broadcast_to(np.arange(P, dtype=BF16), (P, P)).copy()
    per_core = []
    for c in range(ncores):
        sl = slice(c * m_loc, (c + 1) * m_loc)
        head, tail = heads[c], tails[c]
        pc = {}
        # gather indices (layer 0: node id; layer 1: AllGather row)
        pc["hx0"] = t128(head)
        pc["tx0"] = t128(tail)
        pc["hx1"] = t128(invperm[head])
        pc["tx1"] = t128(invperm[tail])
        # slot assignment: combined stream [fwd(=tail-dst), back(=head-dst)]
        dst = np.concatenate([tail, head])
        order = np.argsort(dst, kind="stable")
        ds = dst[order]
        blk = ds >> 7
        starts = np.searchsorted(ds, np.arange(nblocks) << 7)
        idx_in_blk = np.arange(len(ds)) - starts[blk]
        slot = W0[blk] * P + idx_in_blk
        w = slot >> 7
        rrow = (2 * ((w >> 1) * P + (slot & 127)) + (w & 1)).astype(np.int64)
        rmsg = np.zeros(2 * m_loc, np.int64)
        rmsg[order] = rrow
        fsl = np.full(m_pad2, 0, np.int64)
        bsl = np.full(m_pad2, 0, np.int64)
        fsl[:m_loc] = rmsg[:m_loc]
        bsl[:m_loc] = rmsg[m_loc:]
        fsl[m_loc:] = R_slots + np.arange(m_pad2 - m_loc)
        bsl[m_loc:] = R_slots + (m_pad2 - m_loc) + np.arange(m_pad2 - m_loc)
        pc["fsl"] = np.ascontiguousarray(fsl.reshape(a2c, P).T).astype(np.int32)
        pc["bsl"] = np.ascontiguousarray(bsl.reshape(a2c, P).T).astype(np.int32)
        # onehot rel values per slot: [P, T]
        relw = np.full((T, P), 999.0, np.float32)
        relw[slot >> 7, slot & 127] = (ds - (blk << 7)).astype(np.float32)
        pc["relw"] = np.ascontiguousarray(relw.T).astype(BF16)
        # E inputs: pair-row layout + transposed
        E_c = np.zeros((m_pad2, d), np.float32)
        E_c[:m_loc] = E[sl]
        er = E_c.reshape(a2c // 2, 2, P, d).transpose(0, 2, 1, 3).reshape(
            (a2c // 2) * P, 2 * d)
        pc["er0"] = np.ascontiguousarray(er).astype(BF16)
        pc["et0"] = np.ascontiguousarray(E_c.T).astype(BF16)
        # H
        pc["h0"] = H_pad.astype(BF16)
        pc["slot"] = np.zeros((R, d), BF16)
        # owned-shard residual H + inverse counts, pair layout
        op = np.array(own_pairs[c], np.int64)
        nodes_even = (2 * op[:, None] * P + np.arange(P)).reshape(-1)
        nodes_odd = ((2 * op[:, None] + 1) * P + np.arange(P)).reshape(-1)
        hsh = np.zeros((opc * P, 2 * d), np.float32)
        hsh[:, :d] = H_pad[nodes_even]
        hsh[:, d:] = H_pad[nodes_odd]
        pc["hsh2"] = hsh
        ic = np.zeros((P, 2 * opc), np.float32)
        ic[:, 0::2] = inv_cnt[nodes_even].reshape(opc, P).T
        ic[:, 1::2] = inv_cnt[nodes_odd].reshape(opc, P).T
        pc["invc2"] = np.ascontiguousarray(ic)
        pc["iota"] = iota
        for l in range(L):
            for j in range(3):
                pc[f"weu{j}_{l}"] = np.ascontiguousarray(
                    params["W_eu"][l][j * P:(j + 1) * P]).astype(BF16)
            for j in range(2):
                pc[f"wf{j}_{l}"] = np.ascontiguousarray(
                    params["W_fwd"][l][j * P:(j + 1) * P]).astype(BF16)
                pc[f"wb{j}_{l}"] = np.ascontiguousarray(
                    params["W_back"][l][j * P:(j + 1) * P]).astype(BF16)
            for nm, key, isg in (("beu", "b_eu", 0), ("bf", "b_fwd", 0),
                                 ("bb", "b_back", 0), ("ge", "ln_e_g", 1),
                                 ("be", "ln_e_b", 0), ("gh", "ln_h_g", 1),
                                 ("bh", "ln_h_b", 0)):
                if flags[nm]:
                    v = np.broadcast_to(params[key][l], (P, d)).astype(np.float32)
                    pc[f"{nm}_{l}"] = np.ascontiguousarray(
                        np.tile(v, (1, 4)))  # [P, 4d] wide
        per_core.append(pc)
    return meta, per_core


# ---------------------------------------------------------------- program
def _build_program(meta):
    import concourse.bacc as bacc
    import concourse.tile as tile
    from concourse import bass, mybir
    from concourse.bass import IndirectOffsetOnAxis
    from concourse.masks import make_identity

    f32 = mybir.dt.float32
    bf16 = mybir.dt.bfloat16
    i32 = mybir.dt.int32
    Alu = mybir.AluOpType
    Act = mybir.ActivationFunctionType

    d = meta["d"]
    L = meta["L"]
    fl = meta["flags"]
    a2c = meta["a2c"]
    m_pad2 = meta["m_pad2"]
    n_pad = meta["n_pad"]
    npairs = meta["npairs"]
    T = meta["T"]
    R = meta["R"]
    k_b = meta["k_b"]
    W0 = meta["W0"]
    wblk = meta["wblk"]
    CH = meta["CH"]
    opc = meta["opc"]
    shard_n = meta["shard_n"]
    ncores = meta["ncores"]
    NG = a2c // G2
    rg = [list(range(ncores))]

    nc = bacc.Bacc("TRN2", target_bir_lowering=False)

    def apx(base_ap, dims, extra_off=0):
        return bass.AP(tensor=base_ap.tensor, offset=base_ap.offset + extra_off,
                       ap=dims)

    # ---- dram I/O
    h0 = nc.dram_tensor("h0", [n_pad, d], bf16, kind="ExternalInput")
    er0 = nc.dram_tensor("er0", [(a2c // 2) * P, 2 * d], bf16, kind="ExternalInput")
    et0 = nc.dram_tensor("et0", [P, m_pad2], bf16, kind="ExternalInput")
    slot = nc.dram_tensor("slot", [R, d], bf16, kind="ExternalInput")
    hsh2 = nc.dram_tensor("hsh2", [opc * P, 2 * d], f32, kind="ExternalInput")
    invc2_in = nc.dram_tensor("invc2", [P, 2 * opc], f32, kind="ExternalInput")
    iota_in = nc.dram_tensor("iota", [P, P], bf16, kind="ExternalInput")
    hx0_in = nc.dram_tensor("hx0", [P, a2c], i32, kind="ExternalInput")
    tx0_in = nc.dram_tensor("tx0", [P, a2c], i32, kind="ExternalInput")
    hx1_in = nc.dram_tensor("hx1", [P, a2c], i32, kind="ExternalInput")
    tx1_in = nc.dram_tensor("tx1", [P, a2c], i32, kind="ExternalInput")
    fsl_in = nc.dram_tensor("fsl", [P, a2c], i32, kind="ExternalInput")
    bsl_in = nc.dram_tensor("bsl", [P, a2c], i32, kind="ExternalInput")
    relw_in = nc.dram_tensor("relw", [P, T], bf16, kind="ExternalInput")
    hout = nc.dram_tensor("hout", [opc * P, 2 * d], f32, kind="ExternalOutput")
    import os
    DBG = bool(os.environ.get("KERNEL_DEBUG"))
    dbg = {}
    if DBG:
        dbg["er1"] = nc.dram_tensor("dbg_er1", [(a2c // 2) * P, 2 * d], bf16,
                                    kind="ExternalOutput")
        dbg["slot"] = nc.dram_tensor("dbg_slot", [R, d], bf16,
                                     kind="ExternalOutput")
        dbg["agg2"] = nc.dram_tensor("dbg_agg2", [npairs * P, 2 * d], bf16,
                                     kind="ExternalOutput")
        dbg["aggrs"] = nc.dram_tensor("dbg_aggrs", [opc * P, 2 * d], bf16,
                                      kind="ExternalOutput")
        dbg["hn32"] = nc.dram_tensor("dbg_hn32", [opc * P, 2 * d], f32,
                                     kind="ExternalOutput")

    win = {}
    for l in range(L):
        for nm in ("weu0", "weu1", "weu2", "wf0", "wf1", "wb0", "wb1"):
            win[f"{nm}_{l}"] = nc.dram_tensor(f"{nm}_{l}", [P, d], bf16,
                                              kind="ExternalInput")
        for nm in ("beu", "bf", "bb", "ge", "be", "gh", "bh"):
            if fl[nm]:
                win[f"{nm}_{l}"] = nc.dram_tensor(f"{nm}_{l}", [P, 4 * d], f32,
                                                  kind="ExternalInput")

    with tile.TileContext(nc) as tc:
        with (
            tc.tile_pool(name="const", bufs=1) as cp,
            tc.tile_pool(name="dram", bufs=1, space="DRAM") as dp,
            tc.tile_pool(name="sb", bufs=4) as sp,
            tc.tile_pool(name="sbs", bufs=4) as ssp,
            tc.tile_pool(name="ps", bufs=2, space="PSUM") as pp,
        ):
            # ---- persistent DRAM buffers
            hf1 = dp.tile([n_pad, d], bf16, tag="hf1")
            er1 = dp.tile([(a2c // 2) * P, 2 * d], bf16, tag="er1")
            et1 = dp.tile([P, m_pad2], bf16, tag="et1")
            agg2 = dp.tile([npairs * P, 2 * d], bf16, tag="agg2")
            agg_rs = dp.tile([opc * P, 2 * d], bf16, tag="agg_rs")
            hn32 = dp.tile([opc * P, 2 * d], f32, tag="hn32")
            hnbf = dp.tile([shard_n, d], bf16, tag="hnbf")

            # ---- resident SBUF constants
            ident = cp.tile([P, P], bf16, tag="ident")
            make_identity(nc, ident[:])
            eps_t = cp.tile([P, 1], f32, tag="eps")
            nc.vector.memset(eps_t[:], LN_EPS)
            iota_t = cp.tile([P, P], bf16, tag="iota")
            nc.sync.dma_start(out=iota_t[:], in_=iota_in[:])
            idx = {}
            for nm, src in (("hx0", hx0_in), ("tx0", tx0_in), ("hx1", hx1_in),
                            ("tx1", tx1_in), ("fsl", fsl_in), ("bsl", bsl_in)):
                t = cp.tile([P, a2c], i32, tag=nm)
                nc.sync.dma_start(out=t[:], in_=src[:])
                idx[nm] = t
            relw_t = cp.tile([P, T], bf16, tag="relw")
            nc.sync.dma_start(out=relw_t[:], in_=relw_in[:])
            invc_t = cp.tile([P, 2 * opc], f32, tag="invc")
            nc.sync.dma_start(out=invc_t[:], in_=invc2_in[:])
            wt = {}
            for key, src in win.items():
                sh = [P, 4 * d] if any(key.startswith(x) for x in
                                       ("beu", "bf_", "bb", "ge", "be", "gh", "bh")) \
                    and not key.startswith("wf") and not key.startswith("wb") else [P, d]
                t = cp.tile(sh, f32 if sh[1] == 4 * d else bf16, tag=key)
                nc.sync.dma_start(out=t[:], in_=src[:])
                wt[key] = t

            PSTRIDE_ER = [[2 * d, P], [2 * P * d, 4], [1, 2 * d]]

            def ln_wide(z2, nchunks, gk, bk, tag):
                """z2 [P, nchunks*d] f32 -> per-chunk (nmi [P,nchunks], istd)."""
                st6 = ssp.tile([P, 6 * nchunks], f32, tag=f"st6{tag}")
                for cc in range(nchunks):
                    nc.vector.bn_stats(st6[:, 6 * cc:6 * cc + 6],
                                       z2[:, d * cc:d * (cc + 1)])
                st2 = ssp.tile([P, 2 * nchunks], f32, tag=f"st2{tag}")
                for cc in range(nchunks):
                    nc.vector.bn_aggr(st2[:, 2 * cc:2 * cc + 2],
                                      st6[:, 6 * cc:6 * cc + 6])
                std = ssp.tile([P, nchunks], f32, tag=f"std{tag}")
                nc.scalar.activation(
                    std[:], apx(st2[:], [st2[:].ap[0], [2, nchunks]], 1),
                    Act.Sqrt, bias=eps_t[:, 0:1])
                istd = ssp.tile([P, nchunks], f32, tag=f"istd{tag}")
                nc.vector.reciprocal(istd[:], std[:])
                nmi = ssp.tile([P, nchunks], f32, tag=f"nmi{tag}")
                nc.vector.tensor_tensor(
                    out=nmi[:], in0=apx(st2[:], [st2[:].ap[0], [2, nchunks]]),
                    in1=istd[:], op=Alu.mult)
                nc.vector.tensor_scalar_mul(nmi[:], nmi[:], -1.0)
                return nmi, istd

            for l in range(L):
                h_src = h0 if l == 0 else hf1
                er_src = er0 if l == 0 else er1
                et_src = et0 if l == 0 else et1
                hxi = idx["hx0"] if l == 0 else idx["hx1"]
                txi = idx["tx0"] if l == 0 else idx["tx1"]

                # ================= phase A
                n_scat = 2 * a2c
                scat_i = 0

                def slot_out_ap(i):
                    # Disjoint fake tracking ranges break the false WAW chain
                    # between scatters (real rows never collide: each message
                    # owns a unique slot row). First/last scatter per layer
                    # keep the real whole-tensor AP as ordering fences.
                    if i == 0 or i == n_scat - 1:
                        return slot[:]
                    base = slot[:]
                    return bass.AP(tensor=base.tensor, offset=0,
                                   ap=[[d, 1], [1, d]],
                                   dep_tracking_offset=i * d)

                def issue_gathers(g):
                    xh_w = sp.tile([P, G2 * d], bf16, tag="xh")
                    xt_w = sp.tile([P, G2 * d], bf16, tag="xt")
                    for cc in range(G2):
                        col = G2 * g + cc
                        nc.gpsimd.indirect_dma_start(
                            out=xh_w[:, cc * d:(cc + 1) * d], out_offset=None,
                            in_=h_src[:],
                            in_offset=IndirectOffsetOnAxis(
                                ap=hxi[:, col:col + 1], axis=0))
                        nc.gpsimd.indirect_dma_start(
                            out=xt_w[:, cc * d:(cc + 1) * d], out_offset=None,
                            in_=h_src[:],
                            in_offset=IndirectOffsetOnAxis(
                                ap=txi[:, col:col + 1], axis=0))
                    return xh_w, xt_w

                DEPTH = 3
                pref = [issue_gathers(i) for i in range(DEPTH)]
                for g in range(NG):
                    xh_w, xt_w = pref.pop(0)
                    if g + DEPTH < NG:
                        pref.append(issue_gathers(g + DEPTH))
                    er_w = sp.tile([P, G2 * d], bf16, tag="er")
                    nc.sync.dma_start(
                        out=er_w[:],
                        in_=apx(er_src[:, :], PSTRIDE_ER, g * 4 * 2 * P * d))
                    et_w = sp.tile([P, G2 * d], bf16, tag="et")
                    nc.sync.dma_start(out=et_w[:],
                                      in_=et_src[:, G2 * d * g:G2 * d * (g + 1)])
                    mf_w = sp.tile([P, G2 * d], bf16, tag="mf")
                    mb_w = sp.tile([P, G2 * d], bf16, tag="mb")

                    for hh in range(2):
                        o = hh * 4 * d
                        trh = pp.tile([P, 4 * d], bf16, tag="tr")
                        for cc in range(4):
                            nc.tensor.transpose(
                                out=trh[:, cc * d:(cc + 1) * d],
                                in_=xh_w[:, o + cc * d:o + (cc + 1) * d],
                                identity=ident[:])
                        xhT = sp.tile([P, 4 * d], bf16, tag="xhT")
                        nc.scalar.copy(xhT[:], trh[:])
                        trt = pp.tile([P, 4 * d], bf16, tag="tr")
                        for cc in range(4):
                            nc.tensor.transpose(
                                out=trt[:, cc * d:(cc + 1) * d],
                                in_=xt_w[:, o + cc * d:o + (cc + 1) * d],
                                identity=ident[:])
                        xtT = sp.tile([P, 4 * d], bf16, tag="xtT")
                        nc.scalar.copy(xtT[:], trt[:])

                        eu = pp.tile([P, 4 * d], f32, tag="eu")
                        for cc in range(4):
                            sl_ = slice(cc * d, (cc + 1) * d)
                            nc.tensor.matmul(out=eu[:, sl_], lhsT=xhT[:, sl_],
                                             rhs=wt[f"weu0_{l}"][:],
                                             start=True, stop=False)
                            nc.tensor.matmul(out=eu[:, sl_],
                                             lhsT=et_w[:, o + cc * d:o + (cc + 1) * d],
                                             rhs=wt[f"weu1_{l}"][:],
                                             start=False, stop=False)
                            nc.tensor.matmul(out=eu[:, sl_], lhsT=xtT[:, sl_],
                                             rhs=wt[f"weu2_{l}"][:],
                                             start=False, stop=True)
                        if fl["beu"]:
                            eub = sp.tile([P, 4 * d], f32, tag="eub")
                            nc.vector.tensor_add(eub[:], eu[:], wt[f"beu_{l}"][:])
                            zsrc = eub
                        else:
                            zsrc = eu
                        z = sp.tile([P, 4 * d], f32, tag="z")
                        nc.scalar.activation(z[:], zsrc[:], Act.Lrelu,
                                             alpha=LRELU_SLOPE)
                        z2 = sp.tile([P, 4 * d], f32, tag="z2")
                        nc.vector.tensor_tensor(out=z2[:], in0=z[:],
                                                in1=er_w[:, o:o + 4 * d], op=Alu.add)
                        nmi, istd = ln_wide(z2, 4, None, None, "e")
                        enh = sp.tile([P, 4 * d], bf16, tag="enh")
                        for cc in range(4):
                            nc.scalar.activation(
                                enh[:, cc * d:(cc + 1) * d],
                                z2[:, cc * d:(cc + 1) * d], Act.Identity,
                                bias=nmi[:, cc:cc + 1], scale=istd[:, cc:cc + 1])
                        if fl["ge"]:
                            nc.vector.tensor_mul(enh[:], enh[:], wt[f"ge_{l}"][:])
                        if fl["be"]:
                            nc.vector.tensor_add(enh[:], enh[:], wt[f"be_{l}"][:])
                        tre = pp.tile([P, 4 * d], bf16, tag="tr")
                        for cc in range(4):
                            nc.tensor.transpose(
                                out=tre[:, cc * d:(cc + 1) * d],
                                in_=enh[:, cc * d:(cc + 1) * d],
                                identity=ident[:])
                        enT = sp.tile([P, 4 * d], bf16, tag="enT")
                        nc.scalar.copy(enT[:], tre[:])

                        mmf = pp.tile([P, 4 * d], f32, tag="mm")
                        for cc in range(4):
                            sl_ = slice(cc * d, (cc + 1) * d)
                            nc.tensor.matmul(out=mmf[:, sl_], lhsT=xhT[:, sl_],
                                             rhs=wt[f"wf0_{l}"][:],
                                             start=True, stop=False)
                            nc.tensor.matmul(out=mmf[:, sl_], lhsT=enT[:, sl_],
                                             rhs=wt[f"wf1_{l}"][:],
                                             start=False, stop=True)
                        if fl["bf"]:
                            nc.vector.tensor_add(mf_w[:, o:o + 4 * d], mmf[:],
                                                 wt[f"bf_{l}"][:])
                        else:
                            nc.scalar.copy(mf_w[:, o:o + 4 * d], mmf[:])
                        mmb = pp.tile([P, 4 * d], f32, tag="mm")
                        for cc in range(4):
                            sl_ = slice(cc * d, (cc + 1) * d)
                            nc.tensor.matmul(out=mmb[:, sl_], lhsT=xtT[:, sl_],
                                             rhs=wt[f"wb0_{l}"][:],
                                             start=True, stop=False)
                            nc.tensor.matmul(out=mmb[:, sl_], lhsT=enT[:, sl_],
                                             rhs=wt[f"wb1_{l}"][:],
                                             start=False, stop=True)
                        if fl["bb"]:
                            nc.vector.tensor_add(mb_w[:, o:o + 4 * d], mmb[:],
                                                 wt[f"bb_{l}"][:])
                        else:
                            nc.scalar.copy(mb_w[:, o:o + 4 * d], mmb[:])

                        if l == 0:
                            nc.sync.dma_start(
                                out=apx(er1[:, :],
                                        [[2 * d, P], [2 * P * d, 2], [1, 2 * d]],
                                        (4 * g + 2 * hh) * 2 * P * d),
                                in_=enh[:])
                            nc.sync.dma_start(
                                out=et1[:, G2 * d * g + o:G2 * d * g + o + 4 * d],
                                in_=enT[:])

                    for cc in range(G2):
                        col = G2 * g + cc
                        nc.gpsimd.indirect_dma_start(
                            out=slot_out_ap(scat_i), out_offset=IndirectOffsetOnAxis(
                                ap=idx["fsl"][:, col:col + 1], axis=0),
                            in_=mf_w[:, cc * d:(cc + 1) * d], in_offset=None)
                        scat_i += 1
                        nc.gpsimd.indirect_dma_start(
                            out=slot_out_ap(scat_i), out_offset=IndirectOffsetOnAxis(
                                ap=idx["bsl"][:, col:col + 1], axis=0),
                            in_=mb_w[:, cc * d:(cc + 1) * d], in_offset=None)
                        scat_i += 1

                # ================= phase B
                stile = None
                ohw = None
                aggq = None
                cur_q = -1
                ch_idx = 0
                ch_end = CH[0][0] + CH[0][1]
                rs_out_row = 0

                def flush_quad(q):
                    nonlocal aggq
                    qsb = sp.tile([P, 4 * d], bf16, tag="qsb")
                    nc.scalar.copy(qsb[:], aggq[:])
                    nc.sync.dma_start(
                        out=apx(agg2[:, :],
                                [[2 * d, P], [2 * P * d, 2], [1, 2 * d]],
                                2 * q * 2 * P * d),
                        in_=qsb[:])
                    aggq = None

                def maybe_rs(pair_done):
                    nonlocal ch_idx, ch_end, rs_out_row
                    while ch_idx < len(CH) and pair_done >= ch_end:
                        p0, ln_ = CH[ch_idx]
                        out_len = (ln_ // ncores) * P
                        nc.gpsimd.collective_compute(
                            "ReduceScatter", Alu.add, replica_groups=rg,
                            ins=[agg2[p0 * P:(p0 + ln_) * P, :]],
                            outs=[agg_rs[rs_out_row:rs_out_row + out_len, :]])
                        rs_out_row += out_len
                        ch_idx += 1
                        ch_end = CH[ch_idx][0] + CH[ch_idx][1] if ch_idx < len(CH) else 10**9

                for w in range(T):
                    if w % 16 == 0:
                        rb = w // 16
                        stile = sp.tile([P, 16 * d], bf16, tag="stile")
                        nc.sync.dma_start(
                            out=stile[:],
                            in_=apx(slot[:, :], [[2 * d, P], [2 * P * d, 8], [1, 2 * d]],
                                    rb * 8 * 2 * P * d))
                        ohw = sp.tile([P, 16 * d], bf16, tag="ohw")
                        rsl = relw_t[:, 16 * rb:16 * (rb + 1)]
                        nc.vector.tensor_tensor(
                            out=apx(ohw[:], [ohw[:].ap[0], [d, 16], [1, d]]),
                            in0=apx(rsl, [rsl.ap[0], [1, 16], [0, d]]),
                            in1=apx(iota_t[:], [iota_t[:].ap[0], [0, 16], [1, d]]),
                            op=Alu.is_equal)
                    b = int(wblk[w])
                    q = b >> 2
                    if q != cur_q:
                        if cur_q >= 0:
                            flush_quad(cur_q)
                            maybe_rs((cur_q + 1) * 2)
                        aggq = pp.tile([P, 4 * d], f32, tag="agg")
                        cur_q = q
                    wloc = w % 16
                    rhs = stile[:, (wloc >> 1) * 2 * d + (wloc & 1) * d:
                                (wloc >> 1) * 2 * d + (wloc & 1) * d + d]
                    first = (w == W0[b])
                    last = (w == W0[b] + k_b[b] - 1)
                    nc.tensor.matmul(out=aggq[:, (b & 3) * d:((b & 3) + 1) * d],
                                     lhsT=ohw[:, wloc * d:(wloc + 1) * d],
                                     rhs=rhs, start=first, stop=last)
                flush_quad(cur_q)
                maybe_rs(npairs)

                # ================= H update on owned shard
                for qn in range(opc):
                    ag = sp.tile([P, 2 * d], bf16, tag="ag")
                    nc.sync.dma_start(out=ag[:],
                                      in_=agg_rs[qn * P:(qn + 1) * P, :])
                    mn = sp.tile([P, 2 * d], f32, tag="mn")
                    ivs = invc_t[:, 2 * qn:2 * qn + 2]
                    nc.vector.tensor_tensor(
                        out=apx(mn[:], [mn[:].ap[0], [d, 2], [1, d]]),
                        in0=apx(ag[:], [ag[:].ap[0], [d, 2], [1, d]]),
                        in1=apx(ivs, [ivs.ap[0], [1, 2], [0, d]]), op=Alu.mult)
                    zh = sp.tile([P, 2 * d], f32, tag="zh")
                    nc.scalar.activation(zh[:], mn[:], Act.Lrelu, alpha=LRELU_SLOPE)
                    hr = sp.tile([P, 2 * d], f32, tag="hr")
                    hres = hsh2 if l == 0 else hn32
                    nc.sync.dma_start(out=hr[:], in_=hres[qn * P:(qn + 1) * P, :])
                    z2h = sp.tile([P, 2 * d], f32, tag="z2h")
                    nc.vector.tensor_add(z2h[:], zh[:], hr[:])
                    nmi, istd = ln_wide(z2h, 2, None, None, "h")
                    hnt = sp.tile([P, 2 * d], f32, tag="hnt")
                    for cc in range(2):
                        nc.scalar.activation(
                            hnt[:, cc * d:(cc + 1) * d],
                            z2h[:, cc * d:(cc + 1) * d], Act.Identity,
                            bias=nmi[:, cc:cc + 1], scale=istd[:, cc:cc + 1])
                    if fl["gh"]:
                        nc.vector.tensor_mul(hnt[:], hnt[:], wt[f"gh_{l}"][:, :2 * d])
                    if fl["bh"]:
                        nc.vector.tensor_add(hnt[:], hnt[:], wt[f"bh_{l}"][:, :2 * d])
                    tgt = hn32 if l < L - 1 else hout
                    nc.sync.dma_start(out=tgt[qn * P:(qn + 1) * P, :], in_=hnt[:])
                    if l < L - 1:
                        hnb = sp.tile([P, 2 * d], bf16, tag="hnb")
                        nc.scalar.copy(hnb[:], hnt[:])
                        nc.sync.dma_start(
                            out=apx(hnbf[:, :], [[d, P], [P * d, 2], [1, d]],
                                    qn * 2 * P * d),
                            in_=hnb[:])

                if l < L - 1:
                    nc.gpsimd.collective_compute(
                        "AllGather", Alu.bypass, replica_groups=rg,
                        ins=[hnbf[:, :]], outs=[hf1[:, :]])

                if DBG and l == 0:
                    nc.sync.dma_start(out=dbg["er1"][:, :], in_=er1[:, :])
                    nc.sync.dma_start(out=dbg["slot"][:, :], in_=slot[:, :])
                    nc.sync.dma_start(out=dbg["agg2"][:, :], in_=agg2[:, :])
                    nc.sync.dma_start(out=dbg["aggrs"][:, :], in_=agg_rs[:, :])
                    nc.sync.dma_start(out=dbg["hn32"][:, :], in_=hn32[:, :])

    nc.compile()
    return nc


# ---------------------------------------------------------------- entry
def kernel(H, E, ht, queries=None, **params):
    H = np.asarray(H, np.float32)
    E = np.asarray(E, np.float32)
    ht = np.asarray(ht)
    params = {k: np.asarray(v, np.float32) for k, v in params.items()}
    ncores = 8

    meta, per_core = _prep_host(H, E, ht, params, ncores)
    nc = _build_program(meta)

    import os
    from concourse.bass_utils import run_bass_kernel_spmd
    trace = bool(os.environ.get("KERNEL_TRACE"))
    res = run_bass_kernel_spmd(nc, per_core, core_ids=list(range(ncores)),
                               trace=trace)
    global LAST_EXEC_NS
    LAST_EXEC_NS = res.exec_time_ns
    if trace:
        print(f"HW exec time: {res.exec_time_ns} ns")

    n, d = meta["n"], meta["d"]
    out = np.zeros((meta["n_pad"], d), np.float32)
    for c in range(ncores):
        ho = np.asarray(res.results[c]["hout"], np.float32)
        for qn, gp in enumerate(meta["own_pairs"][c]):
            out[2 * gp * P:(2 * gp + 1) * P] = ho[qn * P:(qn + 1) * P, :d]
            out[(2 * gp + 1) * P:(2 * gp + 2) * P] = ho[qn * P:(qn + 1) * P, d:]
    return np.ascontiguousarray(out[:n])


LAST_EXEC_NS = None


# revision 17
# speedup vs baseline: 1.1529x; 1.1529x over previous
"""KGCompletionGNN Trainium2 kernel (8 NeuronCores, SPMD edge-sharding).

v2: bf16 matmuls (4x PE), batched wide DMAs (amortize SWDGE/HWDGE fixed
costs), message scatter into a block-padded pair-interleaved slot buffer
(full-rate contiguous phase-B reads, no phase-B gathers), pair-interleaved
bf16 aggregate with chunked ReduceScatter, wide vector/scalar ops.

Strategy (per sharding hint): shard edges across 8 cores, replicate H.
Per layer:
  Phase A (dbl-groups of 8 edge-chunks): one batched indirect-DMA gather for
    H[head]/H[tail], wide PE transposes, bf16 edge-update matmuls + LayerNorm
    -> E_new; message matmuls -> indirect-scatter bf16 messages into a
    host-zeroed, destination-sorted slot buffer (layout pairs two 128-slot
    windows per 512B DRAM row so phase B reads at full DMA rate).
  Phase B: contiguous reads of slot window-pairs, wide onehot build
    (one is_equal per 16 windows), onehot matmuls accumulate per-node-block
    PSUM quads -> pair-interleaved partial agg [n_pad, d] bf16.
  Chunked ReduceScatter -> per-core node shard; mean + leaky-relu + residual
  + LayerNorm -> H shard; AllGather bf16 H between layers.
Host does index preprocessing only (sharding, sorting, slot schedule).
"""

import sys

sys.path.insert(0, "/opt/trn_rl_repo")

import numpy as np
import ml_dtypes

BF16 = ml_dtypes.bfloat16
P = 128
G2 = 8          # chunks per dbl-group (phase A)
LRELU_SLOPE = 0.01
LN_EPS = 1e-5


# ---------------------------------------------------------------- host prep
def _prep_host(H, E, ht, params, ncores):
    n, d = H.shape
    m = E.shape[0]
    assert d == P
    m_loc = m // ncores
    a2c = -(-m_loc // (P * G2)) * G2            # chunks, multiple of G2
    m_pad2 = a2c * P
    n_pad = -(-n // (ncores * 2 * P)) * (ncores * 2 * P)
    nblocks = n_pad // P
    npairs = nblocks // 2
    L = params["W_eu"].shape[0]

    # ---- flags (biases / ln affine)
    flags = dict(
        beu=bool(np.any(params["b_eu"])), bf=bool(np.any(params["b_fwd"])),
        bb=bool(np.any(params["b_back"])),
        ge=bool(np.any(params["ln_e_g"] != 1)), be=bool(np.any(params["ln_e_b"])),
        gh=bool(np.any(params["ln_h_g"] != 1)), bh=bool(np.any(params["ln_h_b"])),
    )

    # ---- global slot schedule: combined fwd+back messages, dst-sorted,
    # per-block window counts maxed over cores (SPMD-uniform trip counts).
    heads = [ht[c * m_loc:(c + 1) * m_loc, 0].astype(np.int64) for c in range(ncores)]
    tails = [ht[c * m_loc:(c + 1) * m_loc, 1].astype(np.int64) for c in range(ncores)]
    counts = np.zeros((ncores, nblocks), np.int64)
    for c in range(ncores):
        dst = np.concatenate([tails[c], heads[c]])
        counts[c] = np.bincount(dst >> 7, minlength=nblocks)[:nblocks]
    k_b = np.maximum(1, -(-counts.max(axis=0) // P))
    T = int(k_b.sum())
    if T % 16:
        k_b[-1] += 16 - T % 16
        T += 16 - T % 16
    W0 = np.zeros(nblocks + 1, np.int64)
    W0[1:] = np.cumsum(k_b)
    wblk = np.repeat(np.arange(nblocks), k_b)     # block of each window
    R_slots = T * P
    n_trash = 2 * (m_pad2 - m_loc)
    R = R_slots + n_trash

    # slot s -> element-row r (pair-interleaved window layout)
    def r_of_slot(s):
        w = s >> 7
        return 2 * ((w >> 1) * P + (s & 127)) + (w & 1)

    # ---- RS chunking / node ownership
    CH = []
    base = 0
    csz = max(ncores, (npairs // 4) // ncores * ncores)
    sizes = [csz] * 3 + [npairs - 3 * csz] if npairs > 4 * ncores else [npairs]
    for ln_ in sizes:
        CH.append((base, ln_))
        base += ln_
    own_pairs = [[] for _ in range(ncores)]
    for (p0, ln_) in CH:
        per = ln_ // ncores
        for c in range(ncores):
            own_pairs[c].extend(range(p0 + c * per, p0 + (c + 1) * per))
    opc = len(own_pairs[0])                        # owned pairs per core
    shard_n = opc * 2 * P
    invperm = np.zeros(n_pad, np.int64)
    row = 0
    for c in range(ncores):
        for gp in own_pairs[c]:
            invperm[2 * gp * P:(2 * gp + 2) * P] = np.arange(row, row + 2 * P)
            row += 2 * P

    # ---- global count normalization
    cnt = (np.bincount(ht[:, 1], minlength=n_pad)
           + np.bincount(ht[:, 0], minlength=n_pad)).astype(np.float32)
    inv_cnt = (1.0 / np.maximum(cnt, 1.0)).astype(np.float32)

    H_pad = np.zeros((n_pad, d), np.float32)
    H_pad[:n] = H

    meta = dict(
        n=n, d=d, m=m, m_loc=m_loc, a2c=a2c, m_pad2=m_pad2, n_pad=n_pad,
        nblocks=nblocks, npairs=npairs, L=L, T=T, R=R, ncores=ncores,
        k_b=k_b, W0=W0, wblk=wblk, CH=CH, opc=opc, shard_n=shard_n,
        flags=flags, own_pairs=own_pairs,
    )

    def t128(ix, fill=0):
        out = np.full(m_pad2, fill, np.int64)
        out[:len(ix)] = ix
        return np.ascontiguousarray(out.reshape(a2c, P).T).astype(np.int32)

    iota = np.broadcast_to(np.arange(P, dtype=BF16), (P, P)).copy()
    per_core = []
    for c in range(ncores):
        sl = slice(c * m_loc, (c + 1) * m_loc)
        head, tail = heads[c], tails[c]
        pc = {}
        # gather indices (layer 0: node id; layer 1: AllGather row)
        pc["hx0"] = t128(head)
        pc["tx0"] = t128(tail)
        pc["hx1"] = t128(invperm[head])
        pc["tx1"] = t128(invperm[tail])
        # slot assignment: combined stream [fwd(=tail-dst), back(=head-dst)]
        dst = np.concatenate([tail, head])
        order = np.argsort(dst, kind="stable")
        ds = dst[order]
        blk = ds >> 7
        starts = np.searchsorted(ds, np.arange(nblocks) << 7)
        idx_in_blk = np.arange(len(ds)) - starts[blk]
        slot = W0[blk] * P + idx_in_blk
        w = slot >> 7
        rrow = (2 * ((w >> 1) * P + (slot & 127)) + (w & 1)).astype(np.int64)
        rmsg = np.zeros(2 * m_loc, np.int64)
        rmsg[order] = rrow
        fsl = np.full(m_pad2, 0, np.int64)
        bsl = np.full(m_pad2, 0, np.int64)
        fsl[:m_loc] = rmsg[:m_loc]
        bsl[:m_loc] = rmsg[m_loc:]
        fsl[m_loc:] = R_slots + np.arange(m_pad2 - m_loc)
        bsl[m_loc:] = R_slots + (m_pad2 - m_loc) + np.arange(m_pad2 - m_loc)
        pc["fsl"] = np.ascontiguousarray(fsl.reshape(a2c, P).T).astype(np.int32)
        pc["bsl"] = np.ascontiguousarray(bsl.reshape(a2c, P).T).astype(np.int32)
        # onehot rel values per slot: [P, T]
        relw = np.full((T, P), 999.0, np.float32)
        relw[slot >> 7, slot & 127] = (ds - (blk << 7)).astype(np.float32)
        pc["relw"] = np.ascontiguousarray(relw.T).astype(BF16)
        # E inputs: pair-row layout + transposed
        E_c = np.zeros((m_pad2, d), np.float32)
        E_c[:m_loc] = E[sl]
        er = E_c.reshape(a2c // 2, 2, P, d).transpose(0, 2, 1, 3).reshape(
            (a2c // 2) * P, 2 * d)
        pc["er0"] = np.ascontiguousarray(er).astype(BF16)
        pc["et0"] = np.ascontiguousarray(E_c.T).astype(BF16)
        # H
        pc["h0"] = H_pad.astype(BF16)
        pc["slot"] = np.zeros((R, d), BF16)
        # owned-shard residual H + inverse counts, pair layout
        op = np.array(own_pairs[c], np.int64)
        nodes_even = (2 * op[:, None] * P + np.arange(P)).reshape(-1)
        nodes_odd = ((2 * op[:, None] + 1) * P + np.arange(P)).reshape(-1)
        hsh = np.zeros((opc * P, 2 * d), np.float32)
        hsh[:, :d] = H_pad[nodes_even]
        hsh[:, d:] = H_pad[nodes_odd]
        pc["hsh2"] = hsh
        ic = np.zeros((P, 2 * opc), np.float32)
        ic[:, 0::2] = inv_cnt[nodes_even].reshape(opc, P).T
        ic[:, 1::2] = inv_cnt[nodes_odd].reshape(opc, P).T
        pc["invc2"] = np.ascontiguousarray(ic)
        pc["iota"] = iota
        for l in range(L):
            for j in range(3):
                pc[f"weu{j}_{l}"] = np.ascontiguousarray(
                    params["W_eu"][l][j * P:(j + 1) * P]).astype(BF16)
            for j in range(2):
                pc[f"wf{j}_{l}"] = np.ascontiguousarray(
                    params["W_fwd"][l][j * P:(j + 1) * P]).astype(BF16)
                pc[f"wb{j}_{l}"] = np.ascontiguousarray(
                    params["W_back"][l][j * P:(j + 1) * P]).astype(BF16)
            for nm, key, isg in (("beu", "b_eu", 0), ("bf", "b_fwd", 0),
                                 ("bb", "b_back", 0), ("ge", "ln_e_g", 1),
                                 ("be", "ln_e_b", 0), ("gh", "ln_h_g", 1),
                                 ("bh", "ln_h_b", 0)):
                if flags[nm]:
                    v = np.broadcast_to(params[key][l], (P, d)).astype(np.float32)
                    pc[f"{nm}_{l}"] = np.ascontiguousarray(
                        np.tile(v, (1, 4)))  # [P, 4d] wide
        per_core.append(pc)
    return meta, per_core


# ---------------------------------------------------------------- program
def _build_program(meta):
    import concourse.bacc as bacc
    import concourse.tile as tile
    from concourse import bass, mybir
    from concourse.bass import IndirectOffsetOnAxis
    from concourse.masks import make_identity

    f32 = mybir.dt.float32
    bf16 = mybir.dt.bfloat16
    i32 = mybir.dt.int32
    Alu = mybir.AluOpType
    Act = mybir.ActivationFunctionType

    d = meta["d"]
    L = meta["L"]
    fl = meta["flags"]
    a2c = meta["a2c"]
    m_pad2 = meta["m_pad2"]
    n_pad = meta["n_pad"]
    npairs = meta["npairs"]
    T = meta["T"]
    R = meta["R"]
    k_b = meta["k_b"]
    W0 = meta["W0"]
    wblk = meta["wblk"]
    CH = meta["CH"]
    opc = meta["opc"]
    shard_n = meta["shard_n"]
    ncores = meta["ncores"]
    NG = a2c // G2
    rg = [list(range(ncores))]

    nc = bacc.Bacc("TRN2", target_bir_lowering=False)

    def apx(base_ap, dims, extra_off=0):
        return bass.AP(tensor=base_ap.tensor, offset=base_ap.offset + extra_off,
                       ap=dims)

    # ---- dram I/O
    h0 = nc.dram_tensor("h0", [n_pad, d], bf16, kind="ExternalInput")
    er0 = nc.dram_tensor("er0", [(a2c // 2) * P, 2 * d], bf16, kind="ExternalInput")
    et0 = nc.dram_tensor("et0", [P, m_pad2], bf16, kind="ExternalInput")
    slot = nc.dram_tensor("slot", [R, d], bf16, kind="ExternalInput")
    hsh2 = nc.dram_tensor("hsh2", [opc * P, 2 * d], f32, kind="ExternalInput")
    invc2_in = nc.dram_tensor("invc2", [P, 2 * opc], f32, kind="ExternalInput")
    iota_in = nc.dram_tensor("iota", [P, P], bf16, kind="ExternalInput")
    hx0_in = nc.dram_tensor("hx0", [P, a2c], i32, kind="ExternalInput")
    tx0_in = nc.dram_tensor("tx0", [P, a2c], i32, kind="ExternalInput")
    hx1_in = nc.dram_tensor("hx1", [P, a2c], i32, kind="ExternalInput")
    tx1_in = nc.dram_tensor("tx1", [P, a2c], i32, kind="ExternalInput")
    fsl_in = nc.dram_tensor("fsl", [P, a2c], i32, kind="ExternalInput")
    bsl_in = nc.dram_tensor("bsl", [P, a2c], i32, kind="ExternalInput")
    relw_in = nc.dram_tensor("relw", [P, T], bf16, kind="ExternalInput")
    hout = nc.dram_tensor("hout", [opc * P, 2 * d], f32, kind="ExternalOutput")
    import os
    DBG = bool(os.environ.get("KERNEL_DEBUG"))
    dbg = {}
    if DBG:
        dbg["er1"] = nc.dram_tensor("dbg_er1", [(a2c // 2) * P, 2 * d], bf16,
                                    kind="ExternalOutput")
        dbg["slot"] = nc.dram_tensor("dbg_slot", [R, d], bf16,
                                     kind="ExternalOutput")
        dbg["agg2"] = nc.dram_tensor("dbg_agg2", [npairs * P, 2 * d], bf16,
                                     kind="ExternalOutput")
        dbg["aggrs"] = nc.dram_tensor("dbg_aggrs", [opc * P, 2 * d], bf16,
                                      kind="ExternalOutput")
        dbg["hn32"] = nc.dram_tensor("dbg_hn32", [opc * P, 2 * d], f32,
                                     kind="ExternalOutput")

    win = {}
    for l in range(L):
        for nm in ("weu0", "weu1", "weu2", "wf0", "wf1", "wb0", "wb1"):
            win[f"{nm}_{l}"] = nc.dram_tensor(f"{nm}_{l}", [P, d], bf16,
                                              kind="ExternalInput")
        for nm in ("beu", "bf", "bb", "ge", "be", "gh", "bh"):
            if fl[nm]:
                win[f"{nm}_{l}"] = nc.dram_tensor(f"{nm}_{l}", [P, 4 * d], f32,
                                                  kind="ExternalInput")

    with tile.TileContext(nc) as tc:
        with (
            tc.tile_pool(name="const", bufs=1) as cp,
            tc.tile_pool(name="dram", bufs=1, space="DRAM") as dp,
            tc.tile_pool(name="sb", bufs=4) as sp,
            tc.tile_pool(name="sbs", bufs=4) as ssp,
            tc.tile_pool(name="ps", bufs=2, space="PSUM") as pp,
        ):
            # ---- persistent DRAM buffers
            hf1 = dp.tile([n_pad, d], bf16, tag="hf1")
            er1 = dp.tile([(a2c // 2) * P, 2 * d], bf16, tag="er1")
            et1 = dp.tile([P, m_pad2], bf16, tag="et1")
            agg2 = dp.tile([npairs * P, 2 * d], bf16, tag="agg2")
            agg_rs = dp.tile([opc * P, 2 * d], bf16, tag="agg_rs")
            hn32 = dp.tile([opc * P, 2 * d], f32, tag="hn32")
            hnbf = dp.tile([shard_n, d], bf16, tag="hnbf")

            # ---- resident SBUF constants
            ident = cp.tile([P, P], bf16, tag="ident")
            make_identity(nc, ident[:])
            eps_t = cp.tile([P, 1], f32, tag="eps")
            nc.vector.memset(eps_t[:], LN_EPS)
            iota_t = cp.tile([P, P], bf16, tag="iota")
            nc.sync.dma_start(out=iota_t[:], in_=iota_in[:])
            idx = {}
            for nm, src in (("hx0", hx0_in), ("tx0", tx0_in), ("hx1", hx1_in),
                            ("tx1", tx1_in), ("fsl", fsl_in), ("bsl", bsl_in)):
                t = cp.tile([P, a2c], i32, tag=nm)
                nc.sync.dma_start(out=t[:], in_=src[:])
                idx[nm] = t
            relw_t = cp.tile([P, T], bf16, tag="relw")
            nc.sync.dma_start(out=relw_t[:], in_=relw_in[:])
            invc_t = cp.tile([P, 2 * opc], f32, tag="invc")
            nc.sync.dma_start(out=invc_t[:], in_=invc2_in[:])
            wt = {}
            for key, src in win.items():
                sh = [P, 4 * d] if any(key.startswith(x) for x in
                                       ("beu", "bf_", "bb", "ge", "be", "gh", "bh")) \
                    and not key.startswith("wf") and not key.startswith("wb") else [P, d]
                t = cp.tile(sh, f32 if sh[1] == 4 * d else bf16, tag=key)
                nc.sync.dma_start(out=t[:], in_=src[:])
                wt[key] = t

            PSTRIDE_ER = [[2 * d, P], [2 * P * d, 4], [1, 2 * d]]

            def ln_wide(z2, nchunks, gk, bk, tag):
                """z2 [P, nchunks*d] f32 -> per-chunk (nmi [P,nchunks], istd)."""
                st6 = ssp.tile([P, 6 * nchunks], f32, tag=f"st6{tag}")
                for cc in range(nchunks):
                    nc.vector.bn_stats(st6[:, 6 * cc:6 * cc + 6],
                                       z2[:, d * cc:d * (cc + 1)])
                st2 = ssp.tile([P, 2 * nchunks], f32, tag=f"st2{tag}")
                for cc in range(nchunks):
                    nc.vector.bn_aggr(st2[:, 2 * cc:2 * cc + 2],
                                      st6[:, 6 * cc:6 * cc + 6])
                std = ssp.tile([P, nchunks], f32, tag=f"std{tag}")
                nc.scalar.activation(
                    std[:], apx(st2[:], [st2[:].ap[0], [2, nchunks]], 1),
                    Act.Sqrt, bias=eps_t[:, 0:1])
                istd = ssp.tile([P, nchunks], f32, tag=f"istd{tag}")
                nc.vector.reciprocal(istd[:], std[:])
                nmi = ssp.tile([P, nchunks], f32, tag=f"nmi{tag}")
                nc.vector.tensor_tensor(
                    out=nmi[:], in0=apx(st2[:], [st2[:].ap[0], [2, nchunks]]),
                    in1=istd[:], op=Alu.mult)
                nc.vector.tensor_scalar_mul(nmi[:], nmi[:], -1.0)
                return nmi, istd

            for l in range(L):
                h_src = h0 if l == 0 else hf1
                er_src = er0 if l == 0 else er1
                et_src = et0 if l == 0 else et1
                hxi = idx["hx0"] if l == 0 else idx["hx1"]
                txi = idx["tx0"] if l == 0 else idx["tx1"]

                # ================= phase A
                n_scat = 2 * a2c
                scat_i = 0

                def slot_out_ap(i):
                    # Disjoint fake tracking ranges break the false WAW chain
                    # between scatters (real rows never collide: each message
                    # owns a unique slot row). First/last scatter per layer
                    # keep the real whole-tensor AP as ordering fences.
                    if i == 0 or i == n_scat - 1:
                        return slot[:]
                    base = slot[:]
                    return bass.AP(tensor=base.tensor, offset=0,
                                   ap=[[d, 1], [1, d]],
                                   dep_tracking_offset=i * d)

                def issue_gathers(g):
                    xh_w = sp.tile([P, G2 * d], bf16, tag="xh")
                    xt_w = sp.tile([P, G2 * d], bf16, tag="xt")
                    for cc in range(G2):
                        col = G2 * g + cc
                        nc.gpsimd.indirect_dma_start(
                            out=xh_w[:, cc * d:(cc + 1) * d], out_offset=None,
                            in_=h_src[:],
                            in_offset=IndirectOffsetOnAxis(
                                ap=hxi[:, col:col + 1], axis=0))
                        nc.gpsimd.indirect_dma_start(
                            out=xt_w[:, cc * d:(cc + 1) * d], out_offset=None,
                            in_=h_src[:],
                            in_offset=IndirectOffsetOnAxis(
                                ap=txi[:, col:col + 1], axis=0))
                    return xh_w, xt_w

                pref = [issue_gathers(0), issue_gathers(1)]
                for g in range(NG):
                    xh_w, xt_w = pref.pop(0)
                    if g + 2 < NG:
                        pref.append(issue_gathers(g + 2))
                    er_w = sp.tile([P, G2 * d], bf16, tag="er")
                    nc.sync.dma_start(
                        out=er_w[:],
                        in_=apx(er_src[:, :], PSTRIDE_ER, g * 4 * 2 * P * d))
                    et_w = sp.tile([P, G2 * d], bf16, tag="et")
                    nc.sync.dma_start(out=et_w[:],
                                      in_=et_src[:, G2 * d * g:G2 * d * (g + 1)])
                    mf_w = sp.tile([P, G2 * d], bf16, tag="mf")
                    mb_w = sp.tile([P, G2 * d], bf16, tag="mb")

                    for hh in range(2):
                        o = hh * 4 * d
                        trh = pp.tile([P, 4 * d], bf16, tag="tr")
                        for cc in range(4):
                            nc.tensor.transpose(
                                out=trh[:, cc * d:(cc + 1) * d],
                                in_=xh_w[:, o + cc * d:o + (cc + 1) * d],
                                identity=ident[:])
                        xhT = sp.tile([P, 4 * d], bf16, tag="xhT")
                        nc.scalar.copy(xhT[:], trh[:])
                        trt = pp.tile([P, 4 * d], bf16, tag="tr")
                        for cc in range(4):
                            nc.tensor.transpose(
                                out=trt[:, cc * d:(cc + 1) * d],
                                in_=xt_w[:, o + cc * d:o + (cc + 1) * d],
                                identity=ident[:])
                        xtT = sp.tile([P, 4 * d], bf16, tag="xtT")
                        nc.scalar.copy(xtT[:], trt[:])

                        eu = pp.tile([P, 4 * d], f32, tag="eu")
                        for cc in range(4):
                            sl_ = slice(cc * d, (cc + 1) * d)
                            nc.tensor.matmul(out=eu[:, sl_], lhsT=xhT[:, sl_],
                                             rhs=wt[f"weu0_{l}"][:],
                                             start=True, stop=False)
                            nc.tensor.matmul(out=eu[:, sl_],
                                             lhsT=et_w[:, o + cc * d:o + (cc + 1) * d],
                                             rhs=wt[f"weu1_{l}"][:],
                                             start=False, stop=False)
                            nc.tensor.matmul(out=eu[:, sl_], lhsT=xtT[:, sl_],
                                             rhs=wt[f"weu2_{l}"][:],
                                             start=False, stop=True)
                        if fl["beu"]:
                            eub = sp.tile([P, 4 * d], f32, tag="eub")
                            nc.vector.tensor_add(eub[:], eu[:], wt[f"beu_{l}"][:])
                            zsrc = eub
                        else:
                            zsrc = eu
                        z = sp.tile([P, 4 * d], f32, tag="z")
                        nc.scalar.activation(z[:], zsrc[:], Act.Lrelu,
                                             alpha=LRELU_SLOPE)
                        z2 = sp.tile([P, 4 * d], f32, tag="z2")
                        nc.vector.tensor_tensor(out=z2[:], in0=z[:],
                                                in1=er_w[:, o:o + 4 * d], op=Alu.add)
                        nmi, istd = ln_wide(z2, 4, None, None, "e")
                        enh = sp.tile([P, 4 * d], bf16, tag="enh")
                        for cc in range(4):
                            nc.scalar.activation(
                                enh[:, cc * d:(cc + 1) * d],
                                z2[:, cc * d:(cc + 1) * d], Act.Identity,
                                bias=nmi[:, cc:cc + 1], scale=istd[:, cc:cc + 1])
                        if fl["ge"]:
                            nc.vector.tensor_mul(enh[:], enh[:], wt[f"ge_{l}"][:])
                        if fl["be"]:
                            nc.vector.tensor_add(enh[:], enh[:], wt[f"be_{l}"][:])
                        tre = pp.tile([P, 4 * d], bf16, tag="tr")
                        for cc in range(4):
                            nc.tensor.transpose(
                                out=tre[:, cc * d:(cc + 1) * d],
                                in_=enh[:, cc * d:(cc + 1) * d],
                                identity=ident[:])
                        enT = sp.tile([P, 4 * d], bf16, tag="enT")
                        nc.scalar.copy(enT[:], tre[:])

                        mmf = pp.tile([P, 4 * d], f32, tag="mm")
                        for cc in range(4):
                            sl_ = slice(cc * d, (cc + 1) * d)
                            nc.tensor.matmul(out=mmf[:, sl_], lhsT=xhT[:, sl_],
                                             rhs=wt[f"wf0_{l}"][:],
                                             start=True, stop=False)
                            nc.tensor.matmul(out=mmf[:, sl_], lhsT=enT[:, sl_],
                                             rhs=wt[f"wf1_{l}"][:],
                                             start=False, stop=True)
                        if fl["bf"]:
                            nc.vector.tensor_add(mf_w[:, o:o + 4 * d], mmf[:],
                                                 wt[f"bf_{l}"][:])
                        else:
                            nc.scalar.copy(mf_w[:, o:o + 4 * d], mmf[:])
                        mmb = pp.tile([P, 4 * d], f32, tag="mm")
                        for cc in range(4):
                            sl_ = slice(cc * d, (cc + 1) * d)
                            nc.tensor.matmul(out=mmb[:, sl_], lhsT=xtT[:, sl_],
                                             rhs=wt[f"wb0_{l}"][:],
                                             start=True, stop=False)
                            nc.tensor.matmul(out=mmb[:, sl_], lhsT=enT[:, sl_],
                                             rhs=wt[f"wb1_{l}"][:],
                                             start=False, stop=True)
                        if fl["bb"]:
                            nc.vector.tensor_add(mb_w[:, o:o + 4 * d], mmb[:],
                                                 wt[f"bb_{l}"][:])
                        else:
                            nc.scalar.copy(mb_w[:, o:o + 4 * d], mmb[:])

                        if l == 0:
                            nc.sync.dma_start(
                                out=apx(er1[:, :],
                                        [[2 * d, P], [2 * P * d, 2], [1, 2 * d]],
                                        (4 * g + 2 * hh) * 2 * P * d),
                                in_=enh[:])
                            nc.sync.dma_start(
                                out=et1[:, G2 * d * g + o:G2 * d * g + o + 4 * d],
                                in_=enT[:])

                    for cc in range(G2):
                        col = G2 * g + cc
                        nc.gpsimd.indirect_dma_start(
                            out=slot_out_ap(scat_i), out_offset=IndirectOffsetOnAxis(
                                ap=idx["fsl"][:, col:col + 1], axis=0),
                            in_=mf_w[:, cc * d:(cc + 1) * d], in_offset=None)
                        scat_i += 1
                        nc.gpsimd.indirect_dma_start(
                            out=slot_out_ap(scat_i), out_offset=IndirectOffsetOnAxis(
                                ap=idx["bsl"][:, col:col + 1], axis=0),
                            in_=mb_w[:, cc * d:(cc + 1) * d], in_offset=None)
                        scat_i += 1

                # ================= phase B
                stile = None
                ohw = None
                aggq = None
                cur_q = -1
                ch_idx = 0
                ch_end = CH[0][0] + CH[0][1]
                rs_out_row = 0

                def flush_quad(q):
                    nonlocal aggq
                    qsb = sp.tile([P, 4 * d], bf16, tag="qsb")
                    nc.scalar.copy(qsb[:], aggq[:])
                    nc.sync.dma_start(
                        out=apx(agg2[:, :],
                                [[2 * d, P], [2 * P * d, 2], [1, 2 * d]],
                                2 * q * 2 * P * d),
                        in_=qsb[:])
                    aggq = None

                def maybe_rs(pair_done):
                    nonlocal ch_idx, ch_end, rs_out_row
                    while ch_idx < len(CH) and pair_done >= ch_end:
                        p0, ln_ = CH[ch_idx]
                        out_len = (ln_ // ncores) * P
                        nc.gpsimd.collective_compute(
                            "ReduceScatter", Alu.add, replica_groups=rg,
                            ins=[agg2[p0 * P:(p0 + ln_) * P, :]],
                            outs=[agg_rs[rs_out_row:rs_out_row + out_len, :]])
                        rs_out_row += out_len
                        ch_idx += 1
                        ch_end = CH[ch_idx][0] + CH[ch_idx][1] if ch_idx < len(CH) else 10**9

                for w in range(T):
                    if w % 16 == 0:
                        rb = w // 16
                        stile = sp.tile([P, 16 * d], bf16, tag="stile")
                        nc.sync.dma_start(
                            out=stile[:],
                            in_=apx(slot[:, :], [[2 * d, P], [2 * P * d, 8], [1, 2 * d]],
                                    rb * 8 * 2 * P * d))
                        ohw = sp.tile([P, 16 * d], bf16, tag="ohw")
                        rsl = relw_t[:, 16 * rb:16 * (rb + 1)]
                        nc.vector.tensor_tensor(
                            out=apx(ohw[:], [ohw[:].ap[0], [d, 16], [1, d]]),
                            in0=apx(rsl, [rsl.ap[0], [1, 16], [0, d]]),
                            in1=apx(iota_t[:], [iota_t[:].ap[0], [0, 16], [1, d]]),
                            op=Alu.is_equal)
                    b = int(wblk[w])
                    q = b >> 2
                    if q != cur_q:
                        if cur_q >= 0:
                            flush_quad(cur_q)
                            maybe_rs((cur_q + 1) * 2)
                        aggq = pp.tile([P, 4 * d], f32, tag="agg")
                        cur_q = q
                    wloc = w % 16
                    rhs = stile[:, (wloc >> 1) * 2 * d + (wloc & 1) * d:
                                (wloc >> 1) * 2 * d + (wloc & 1) * d + d]
                    first = (w == W0[b])
                    last = (w == W0[b] + k_b[b] - 1)
                    nc.tensor.matmul(out=aggq[:, (b & 3) * d:((b & 3) + 1) * d],
                                     lhsT=ohw[:, wloc * d:(wloc + 1) * d],
                                     rhs=rhs, start=first, stop=last)
                flush_quad(cur_q)
                maybe_rs(npairs)

                # ================= H update on owned shard
                for qn in range(opc):
                    ag = sp.tile([P, 2 * d], bf16, tag="ag")
                    nc.sync.dma_start(out=ag[:],
                                      in_=agg_rs[qn * P:(qn + 1) * P, :])
                    mn = sp.tile([P, 2 * d], f32, tag="mn")
                    ivs = invc_t[:, 2 * qn:2 * qn + 2]
                    nc.vector.tensor_tensor(
                        out=apx(mn[:], [mn[:].ap[0], [d, 2], [1, d]]),
                        in0=apx(ag[:], [ag[:].ap[0], [d, 2], [1, d]]),
                        in1=apx(ivs, [ivs.ap[0], [1, 2], [0, d]]), op=Alu.mult)
                    zh = sp.tile([P, 2 * d], f32, tag="zh")
                    nc.scalar.activation(zh[:], mn[:], Act.Lrelu, alpha=LRELU_SLOPE)
                    hr = sp.tile([P, 2 * d], f32, tag="hr")
                    hres = hsh2 if l == 0 else hn32
                    nc.sync.dma_start(out=hr[:], in_=hres[qn * P:(qn + 1) * P, :])
                    z2h = sp.tile([P, 2 * d], f32, tag="z2h")
                    nc.vector.tensor_add(z2h[:], zh[:], hr[:])
                    nmi, istd = ln_wide(z2h, 2, None, None, "h")
                    hnt = sp.tile([P, 2 * d], f32, tag="hnt")
                    for cc in range(2):
                        nc.scalar.activation(
                            hnt[:, cc * d:(cc + 1) * d],
                            z2h[:, cc * d:(cc + 1) * d], Act.Identity,
                            bias=nmi[:, cc:cc + 1], scale=istd[:, cc:cc + 1])
                    if fl["gh"]:
                        nc.vector.tensor_mul(hnt[:], hnt[:], wt[f"gh_{l}"][:, :2 * d])
                    if fl["bh"]:
                        nc.vector.tensor_add(hnt[:], hnt[:], wt[f"bh_{l}"][:, :2 * d])
                    tgt = hn32 if l < L - 1 else hout
                    nc.sync.dma_start(out=tgt[qn * P:(qn + 1) * P, :], in_=hnt[:])
                    if l < L - 1:
                        hnb = sp.tile([P, 2 * d], bf16, tag="hnb")
                        nc.scalar.copy(hnb[:], hnt[:])
                        nc.sync.dma_start(
                            out=apx(hnbf[:, :], [[d, P], [P * d, 2], [1, d]],
                                    qn * 2 * P * d),
                            in_=hnb[:])

                if l < L - 1:
                    nc.gpsimd.collective_compute(
                        "AllGather", Alu.bypass, replica_groups=rg,
                        ins=[hnbf[:, :]], outs=[hf1[:, :]])

                if DBG and l == 0:
                    nc.sync.dma_start(out=dbg["er1"][:, :], in_=er1[:, :])
                    nc.sync.dma_start(out=dbg["slot"][:, :], in_=slot[:, :])
                    nc.sync.dma_start(out=dbg["agg2"][:, :], in_=agg2[:, :])
                    nc.sync.dma_start(out=dbg["aggrs"][:, :], in_=agg_rs[:, :])
                    nc.sync.dma_start(out=dbg["hn32"][:, :], in_=hn32[:, :])

    nc.compile()
    return nc


# ---------------------------------------------------------------- entry
def kernel(H, E, ht, queries=None, **params):
    H = np.asarray(H, np.float32)
    E = np.asarray(E, np.float32)
    ht = np.asarray(ht)
    params = {k: np.asarray(v, np.float32) for k, v in params.items()}
    ncores = 8

    meta, per_core = _prep_host(H, E, ht, params, ncores)
    nc = _build_program(meta)

    import os
    from concourse.bass_utils import run_bass_kernel_spmd
    trace = bool(os.environ.get("KERNEL_TRACE"))
    res = run_bass_kernel_spmd(nc, per_core, core_ids=list(range(ncores)),
                               trace=trace)
    global LAST_EXEC_NS
    LAST_EXEC_NS = res.exec_time_ns
    if trace:
        print(f"HW exec time: {res.exec_time_ns} ns")

    n, d = meta["n"], meta["d"]
    out = np.zeros((meta["n_pad"], d), np.float32)
    for c in range(ncores):
        ho = np.asarray(res.results[c]["hout"], np.float32)
        for qn, gp in enumerate(meta["own_pairs"][c]):
            out[2 * gp * P:(2 * gp + 1) * P] = ho[qn * P:(qn + 1) * P, :d]
            out[(2 * gp + 1) * P:(2 * gp + 2) * P] = ho[qn * P:(qn + 1) * P, d:]
    return np.ascontiguousarray(out[:n])


LAST_EXEC_NS = None
